# revision 1
# baseline (speedup 1.0000x reference)
"""Bilateral grid slice+apply on 8 Trainium2 NeuronCores.

Gather-free formulation: the per-pixel trilinear interpolation is expressed
in the hat-function basis  hat(a) = relu(1 - |a|)  and evaluated densely as
matmuls with the (tiny) grid as the stationary operand:

    coeffs[n, z, c] = sum_{y,x} hy(n,y) hx(n,x) * G[y, x, z, c]     (PE, K=256)
    out[n, c3]      = sum_{z,j} hz(n,z) * xt[n,j] * coeffs[n, z, 4c3+j]

Pixels ride the matmul free dimension (512 per tile); hats are built with one
PE broadcast matmul + Abs/Relu activations; the z/affine fold is two
elementwise muls + a final K=96 reduce matmul.

Data parallel: pixels are sharded across the 8 cores; the 16x16x8x12 grid is
replicated (host bakes it into the stationary operands).
"""
import numpy as np
from contextlib import ExitStack

import concourse.bass as bass
import concourse.bacc as bacc
import concourse.mybir as mybir
from concourse import tile
from concourse.bass_utils import run_bass_kernel_spmd

F = 512             # pixels per tile (one fp32 PSUM bank)
NCORES = 8
B, H, W = 4, 1080, 1920
NTOT = B * H * W                  # 8294400
NPC = NTOT // NCORES              # 1036800 per core
T = NPC // F                      # 2025 tiles per core
LUM = (0.2126, 0.7152, 0.0722)

_CACHE = {}


def _make_stationaries(grid):
    g = grid.astype(np.float32)
    stP0 = np.zeros((5, 72), np.float32)      # rows (r,g,b,cx,cy)
    for m in range(16):
        stP0[4, m] = 15.0                     # gy from cy
    for m in range(32, 48):
        stP0[3, m] = 15.0                     # gx from cx
    for m in range(64, 72):
        stP0[0, m] = 7.0 * LUM[0]
        stP0[1, m] = 7.0 * LUM[1]
        stP0[2, m] = 7.0 * LUM[2]
    bias40 = np.zeros((72, 1), np.float32)
    bias40[:16, 0] = -np.arange(16)
    bias40[32:48, 0] = -np.arange(16)
    bias40[64:72, 0] = -np.arange(8)

    stHY = np.zeros((2, 16, 128), np.float32)
    stHX = np.zeros((2, 16, 128), np.float32)
    for p in range(2):
        for m in range(128):
            stHY[p, p * 8 + m // 16, m] = 1.0
            stHX[p, m % 16, m] = 1.0

    stMAIN = np.zeros((2, 128, 96), np.float32)
    for p in range(2):
        for k in range(128):
            stMAIN[p, k, :] = g[p * 8 + k // 16, k % 16].reshape(96)

    stHZ = np.zeros((8, 96), np.float32)
    for z in range(8):
        stHZ[z, z * 12:(z + 1) * 12] = 1.0

    stX = np.zeros((4, 96), np.float32)       # rhs rows (ones, r, g, b)
    for z in range(8):
        for c3 in range(3):
            for j in range(4):
                stX[0 if j == 3 else j + 1, z * 12 + c3 * 4 + j] = 1.0

    stRED = np.zeros((96, 3), np.float32)
    for z in range(8):
        for c3 in range(3):
            for j in range(4):
                stRED[z * 12 + c3 * 4 + j, c3] = 1.0

    return dict(stP0=stP0, bias40=bias40,
                stHYa=stHY[0], stHYb=stHY[1], stHXa=stHX[0], stHXb=stHX[1],
                stMAINa=stMAIN[0], stMAINb=stMAIN[1],
                stHZ=stHZ, stX=stX, stRED=stRED)


def build_kernel(ntiles=T, num_cores=NCORES, reps=1):
    nc = bacc.Bacc("TRN2", target_bir_lowering=False, debug=False,
                   num_devices=num_cores)
    NP = ntiles * F
    f32 = mybir.dt.float32

    in5 = nc.declare_dram_parameter("in5", [5, NP], f32, isOutput=False)
    inx = nc.declare_dram_parameter("inx", [4, NP], f32, isOutput=False)
    decls = {}
    for nm, shp in (("stP0", [5, 72]), ("bias40", [72, 1]),
                    ("stHYa", [16, 128]), ("stHYb", [16, 128]),
                    ("stHXa", [16, 128]), ("stHXb", [16, 128]),
                    ("stMAINa", [128, 96]), ("stMAINb", [128, 96]),
                    ("stHZ", [8, 96]), ("stX", [4, 96]), ("stRED", [96, 3])):
        decls[nm] = nc.declare_dram_parameter(nm, shp, f32, isOutput=False)
    out3 = nc.declare_dram_parameter("out3", [3, NP], f32, isOutput=True)

    with tile.TileContext(nc) as tc:
        with ExitStack() as ctx:
            stp = ctx.enter_context(tc.tile_pool(name="stats", bufs=1))
            sP0 = stp.tile([5, 72], f32, tag="sP0")
            sB40 = stp.tile([72, 1], f32, tag="sB40")
            sHYa = stp.tile([16, 128], f32, tag="sHYa")
            sHYb = stp.tile([16, 128], f32, tag="sHYb")
            sHXa_t = stp.tile([48, 128], f32, tag="sHXa")
            sHXa = sHXa_t[32:48, :]
            sMa = stp.tile([128, 96], f32, tag="sMa")
            sMb = stp.tile([128, 96], f32, tag="sMb")
            sHZ_t = stp.tile([72, 96], f32, tag="sHZ")
            sX_t = stp.tile([36, 96], f32, tag="sX")
            sHZ = sHZ_t[64:72, :]
            sX = sX_t[32:36, :]
            sRED = stp.tile([96, 3], f32, tag="sRED")
            for t_, nm in ((sP0[:], "stP0"), (sB40[:], "bias40"),
                           (sHYa[:], "stHYa"), (sHYb[:], "stHYb"),
                           (sHXa, "stHXa"),
                           (sMa[:], "stMAINa"), (sMb[:], "stMAINb"),
                           (sHZ, "stHZ"), (sX, "stX"), (sRED[:], "stRED")):
                nc.sync.dma_start(t_, decls[nm].ap())

            sb_in = ctx.enter_context(tc.tile_pool(name="sb_in", bufs=3))
            sb_mid = ctx.enter_context(tc.tile_pool(name="sb_mid", bufs=3))
            sb_w = ctx.enter_context(tc.tile_pool(name="sb_w", bufs=2))
            sb_wab = ctx.enter_context(tc.tile_pool(name="sb_wab", bufs=1))
            ps_args = ctx.enter_context(tc.tile_pool(name="ps_args", bufs=1, space="PSUM"))
            ps_rep = ctx.enter_context(tc.tile_pool(name="ps_rep", bufs=2, space="PSUM"))
            ps_rep2 = ctx.enter_context(tc.tile_pool(name="ps_rep2", bufs=2, space="PSUM"))
            ps_cf = ctx.enter_context(tc.tile_pool(name="ps_cf", bufs=1, space="PSUM"))
            ps_zx = ctx.enter_context(tc.tile_pool(name="ps_zx", bufs=1, space="PSUM"))

            G = 6
            for _rep in range(reps):
              for g0 in range(0, ntiles, G):
                gtiles = range(g0, min(g0 + G, ntiles))
                ins, hats_l, Wa_l, Wb_l = {}, {}, {}, {}
                for i in gtiles:      # phase 1: load, hat args, hats
                    IN6 = sb_in.tile([36, F], f32, tag=f"in6_{i%(G+1)}")
                    nc.sync.dma_start(IN6[0:5, :], in5.ap()[:, bass.ts(i, F)])
                    nc.sync.dma_start(IN6[32:36, :], inx.ap()[:, bass.ts(i, F)])
                    ins[i] = IN6
                    argsP = ps_args.tile([72, F], f32, tag="args")
                    nc.tensor.matmul(argsP[:], sP0[:], IN6[0:5, :], start=True, stop=True)
                    tabs = sb_mid.tile([72, F], f32, tag="tabs")
                    nc.scalar.activation(tabs[:], argsP[:],
                                         mybir.ActivationFunctionType.Abs,
                                         bias=sB40[:], scale=1.0)
                    hats = sb_mid.tile([72, F], f32, tag=f"hats_{i%(G+1)}")
                    nc.scalar.activation(hats[:], tabs[:],
                                         mybir.ActivationFunctionType.Relu,
                                         bias=1.0, scale=-1.0)
                    hats_l[i] = hats
                for i in gtiles:      # phase 2: W = hy (x) hx
                    hats = hats_l[i]
                    HYa = ps_rep.tile([128, F], f32, tag="HY")
                    HXa = ps_rep2.tile([128, F], f32, tag="HX")
                    nc.tensor.matmul(HYa[:], sHYa[:], hats[0:16, :], start=True, stop=True)
                    nc.tensor.matmul(HXa[:], sHXa, hats[32:48, :], start=True, stop=True)
                    HXaS = sb_w.tile([128, F], f32, tag="HXaS")
                    nc.scalar.copy(HXaS[:], HXa[:])
                    Wa = sb_wab.tile([128, F], f32, tag=f"Wa_{i%(G+1)}")
                    nc.vector.tensor_tensor(out=Wa[:], in0=HYa[:], in1=HXaS[:],
                                            op=mybir.AluOpType.mult)
                    HYb = ps_rep.tile([128, F], f32, tag="HY")
                    nc.tensor.matmul(HYb[:], sHYb[:], hats[0:16, :], start=True, stop=True)
                    Wb = sb_wab.tile([128, F], f32, tag=f"Wb_{i%(G+1)}")
                    nc.vector.tensor_tensor(out=Wb[:], in0=HYb[:], in1=HXaS[:],
                                            op=mybir.AluOpType.mult)
                    Wa_l[i], Wb_l[i] = Wa, Wb
                for i in gtiles:      # phase 3: mains + z/affine fold + out
                    hats, IN6 = hats_l[i], ins[i]
                    CF = ps_cf.tile([96, F], f32, tag="CF")
                    nc.tensor.matmul(CF[:], sMa[:], Wa_l[i][:], start=True, stop=False)
                    nc.tensor.matmul(CF[:], sMb[:], Wb_l[i][:], start=False, stop=True)
                    HZ96 = ps_zx.tile([96, F], f32, tag="HZ")
                    X96 = ps_cf.tile([96, F], f32, tag="X96")
                    nc.tensor.matmul(HZ96[:], sHZ, hats[64:72, :], start=True, stop=True)
                    nc.tensor.matmul(X96[:], sX, IN6[32:36, :], start=True, stop=True)
                    HZS = sb_w.tile([96, F], f32, tag="HZS")
                    nc.scalar.copy(HZS[:], HZ96[:])
                    HZX = sb_w.tile([96, F], f32, tag="HZX")
                    nc.vector.tensor_tensor(out=HZX[:], in0=X96[:], in1=HZS[:],
                                            op=mybir.AluOpType.mult)
                    M2 = sb_w.tile([96, F], f32, tag="M2")
                    nc.vector.tensor_tensor(out=M2[:], in0=CF[:], in1=HZX[:],
                                            op=mybir.AluOpType.mult)
                    OUT3 = ps_zx.tile([3, F], f32, tag="HZ")
                    nc.tensor.matmul(OUT3[:], sRED[:], M2[:], start=True, stop=True)
                    OUTS = sb_in.tile([3, F], f32, tag="outs")
                    nc.scalar.copy(OUTS[:], OUT3[:])
                    nc.sync.dma_start(out3.ap()[:, bass.ts(i, F)], OUTS[:])

    nc.compile()
    return nc


def kernel(pixels: np.ndarray, coords: np.ndarray, grid: np.ndarray) -> np.ndarray:
    assert pixels.shape == (B, H, W, 3) and coords.shape == (B, H, W, 2)
    p = np.asarray(pixels, np.float32).reshape(-1, 3)
    c = np.asarray(coords, np.float32).reshape(-1, 2)
    r = np.ascontiguousarray(p[:, 0]); g = np.ascontiguousarray(p[:, 1])
    b = np.ascontiguousarray(p[:, 2])
    cx = np.ascontiguousarray(c[:, 0]); cy = np.ascontiguousarray(c[:, 1])
    ones = np.ones(NPC, np.float32)

    stats = _make_stationaries(np.asarray(grid, np.float32))
    in_maps = []
    for cid in range(NCORES):
        s = slice(cid * NPC, (cid + 1) * NPC)
        in5 = np.ascontiguousarray(np.stack([r[s], g[s], b[s], cx[s], cy[s]]))
        inx = np.ascontiguousarray(np.stack([ones, r[s], g[s], b[s]]))
        in_maps.append({"in5": in5, "inx": inx, **stats})

    if "nc" not in _CACHE:
        _CACHE["nc"] = build_kernel()
    nc = _CACHE["nc"]
    res = run_bass_kernel_spmd(nc, in_maps, list(range(NCORES)))
    out = np.concatenate([res.results[cid]["out3"].T for cid in range(NCORES)], 0)
    return np.ascontiguousarray(out.reshape(B, H, W, 3).astype(np.float32))



# revision 6
# speedup vs baseline: 103.5038x; 103.5038x over previous
"""Bilateral grid slice+apply on 8 Trainium2 NeuronCores.

Gather-free hat-basis formulation, v2 (fp16 + DMA broadcasts):

    hat(a) = relu(1 - |a|);  nh = -hat  computed as  min(|a|-1, 0)
    W[cell] = hat_y * hat_x  (256 cells, x-major, split in two 128 halves)
    CF[z*12+k] = sum_cell W G[cell, z, k]         (PE, K=128 x2 accumulate)
    out_c = sum_p RED[p,c] * CF[p] * hz[p] * q[p]  (PE reduce K=96)

Engine split per 512-pixel tile:
  PE   : args (block-diag, N=256), HY one-hot x2, X96 one-hot x2,
         CFa+CFb, OUT reduce x2            (~9 matmuls, all fp16 moving)
  ACT  : Abs(args+bias), HY->SBUF f16 copy, X96->SBUF f16 copy, OUT copy
  DVE  : nh chain (4x mode), Wa, Wb, HZX muls (2x mode), M2 mul
  DMA  : HXa/HXb/HZ96 partition-repeat broadcasts (SBUF->SBUF), IO

Pixels are packed two-blocks-per-tile: partitions [0:6)/[6:12) hold input
channels (r,g,b,1,cy,cx) of pixels [0:256)/[256:512) of the tile, halving
the free dim of the args/abs/nh/out stages.

Data parallel: pixels sharded across 8 cores; the 16x16x8x12 grid is
replicated (host bakes it into the stationary operands).
"""
import numpy as np
from contextlib import ExitStack

import concourse.bass as bass
import concourse.bacc as bacc
import concourse.mybir as mybir
from concourse import tile
from concourse.bass_utils import run_bass_kernel_spmd

F = 512             # pixels per tile
HALF = 256
NCORES = 8
B, H, W = 4, 1080, 1920
NTOT = B * H * W                  # 8294400
NPC = NTOT // NCORES              # 1036800 per core
T = NPC // F                      # 2025 tiles per core
LUM = (0.2126, 0.7152, 0.0722)

_CACHE = {}


def _make_stationaries(grid):
    g = grid.astype(np.float32)
    f16 = np.float16

    # args matmul: in6 rows (r,g,b,1,cy,cx) x2 blocks -> 80 hat args
    stARG = np.zeros((12, 80), np.float32)
    biasARG = np.zeros((80, 1), np.float32)
    for h in range(2):
        for j in range(40):
            m = 40 * h + j
            if j < 16:
                stARG[6 * h + 4, m] = 15.0      # gy = 15*cy
                biasARG[m, 0] = -j
            elif j < 32:
                stARG[6 * h + 5, m] = 15.0      # gx = 15*cx
                biasARG[m, 0] = -(j - 16)
            else:
                for c in range(3):
                    stARG[6 * h + c, m] = 7.0 * LUM[c]   # gz = 7*lum
                biasARG[m, 0] = -(j - 32)

    # HY one-hot: HY[m] = nh[40*h + m%16] (y-hats), h = column half
    stHYe = np.zeros((80, 128), np.float32)
    stHYo = np.zeros((80, 128), np.float32)
    for m in range(128):
        stHYe[m % 16, m] = 1.0
        stHYo[40 + m % 16, m] = 1.0

    # X96 one-hot: X96[p] = IN6[6*h + p%4]  (rows r,g,b,1)
    stXe = np.zeros((12, 96), np.float32)
    stXo = np.zeros((12, 96), np.float32)
    for p in range(96):
        stXe[p % 4, p] = 1.0
        stXo[6 + p % 4, p] = 1.0

    # main grid stationaries, cells x-major: m = 16*x' + y
    Ga = np.zeros((128, 96), np.float32)
    Gb = np.zeros((128, 96), np.float32)
    for m in range(128):
        y, xp = m % 16, m // 16
        Ga[m, :] = g[y, xp].reshape(96)
        Gb[m, :] = g[y, 8 + xp].reshape(96)

    # final reduce (negated to absorb nh_z sign), packed out [6, HALF]
    REDe = np.zeros((96, 6), np.float32)
    REDo = np.zeros((96, 6), np.float32)
    for p in range(96):
        c = (p % 12) // 4
        REDe[p, c] = -1.0
        REDo[p, 3 + c] = -1.0

    return dict(
        stARG=stARG.astype(f16), biasARG=biasARG,
        stHYe=stHYe.astype(f16), stHYo=stHYo.astype(f16),
        stXe=stXe.astype(f16), stXo=stXo.astype(f16),
        Ga=Ga.astype(f16), Gb=Gb.astype(f16),
        REDe=REDe.astype(f16), REDo=REDo.astype(f16),
    )


def build_kernel(ntiles=T, num_cores=NCORES, reps=1, hw_loop=False):
    nc = bacc.Bacc("TRN2", target_bir_lowering=False, debug=False,
                   num_devices=num_cores)
    NH = ntiles * HALF
    f16, f32 = mybir.dt.float16, mybir.dt.float32

    in6 = nc.declare_dram_parameter("in6", [12, NH], f16, isOutput=False)
    decls = {}
    for nm, shp, dt_ in (("stARG", [12, 80], f16), ("biasARG", [80, 1], f32),
                         ("stHYe", [80, 128], f16), ("stHYo", [80, 128], f16),
                         ("stXe", [12, 96], f16), ("stXo", [12, 96], f16),
                         ("Ga", [128, 96], f16), ("Gb", [128, 96], f16),
                         ("REDe", [96, 6], f16), ("REDo", [96, 6], f16)):
        decls[nm] = nc.declare_dram_parameter(nm, shp, dt_, isOutput=False)
    out6 = nc.declare_dram_parameter("out6", [6, NH], f16, isOutput=True)

    with tile.TileContext(nc) as tc:
        with ExitStack() as ctx:
            stp = ctx.enter_context(tc.tile_pool(name="stats", bufs=1))
            st = {}
            for nm, shp, dt_ in (("stARG", [12, 80], f16), ("biasARG", [80, 1], f32),
                                 ("stHYe", [80, 128], f16), ("stHYo", [80, 128], f16),
                                 ("stXe", [12, 96], f16), ("stXo", [12, 96], f16),
                                 ("Ga", [128, 96], f16), ("Gb", [128, 96], f16),
                                 ("REDe", [96, 6], f16), ("REDo", [96, 6], f16)):
                t_ = stp.tile(shp, dt_, tag=nm, name=nm)
                nc.sync.dma_start(t_[:], decls[nm].ap())
                st[nm] = t_

            sb_in = ctx.enter_context(tc.tile_pool(name="sb_in", bufs=1))
            sb_w = ctx.enter_context(tc.tile_pool(name="sb_w", bufs=1))
            sb_s = ctx.enter_context(tc.tile_pool(name="sb_s", bufs=3))
            ps_args = ctx.enter_context(tc.tile_pool(name="ps_args", bufs=2, space="PSUM"))
            ps_hy = ctx.enter_context(tc.tile_pool(name="ps_hy", bufs=2, space="PSUM"))
            ps_x = ctx.enter_context(tc.tile_pool(name="ps_x", bufs=1, space="PSUM"))
            ps_cf = ctx.enter_context(tc.tile_pool(name="ps_cf", bufs=2, space="PSUM"))
            ps_out = ctx.enter_context(tc.tile_pool(name="ps_out", bufs=1, space="PSUM"))

            G = 6
            NB = G + 1

            def body():
                for g0 in range(0, ntiles, G):
                    gtiles = range(g0, min(g0 + G, ntiles))
                    ins, nhs, hx, wab = {}, {}, {}, {}
                    for i in gtiles:      # phase 1: load, args, hats
                        IN6 = sb_in.tile([12, HALF], f16, tag=f"in6_{i % NB}",
                                         bufs=1, name="IN6")
                        nc.sync.dma_start(IN6[:], in6.ap()[:, bass.ts(i, HALF)])
                        ins[i] = IN6
                        argsP = ps_args.tile([80, HALF], f32, tag="args", name="argsP")
                        nc.tensor.matmul(argsP[:], st["stARG"][:], IN6[:],
                                         start=True, stop=True)
                        tabs = sb_s.tile([80, HALF], f16, tag="tabs", name="tabs")
                        nc.scalar.activation(tabs[:], argsP[:],
                                             mybir.ActivationFunctionType.Abs,
                                             bias=st["biasARG"][:], scale=1.0)
                        nh = sb_in.tile([80, HALF], f16, tag=f"nh_{i % NB}",
                                        bufs=1, name="nh")
                        nc.vector.tensor_scalar(out=nh[:], in0=tabs[:],
                                                scalar1=1.0, scalar2=0.0,
                                                op0=mybir.AluOpType.subtract,
                                                op1=mybir.AluOpType.min)
                        nhs[i] = nh
                    for i in gtiles:      # phase 2: broadcasts + W
                        nh = nhs[i]
                        nap = nh[:]
                        ps = nap.ap[0][0]

                        def rep(row, cnt, rep_n):
                            return bass.AP(nap.tensor, nap.offset + row * ps,
                                           [[ps, cnt], [0, rep_n], [1, HALF]])

                        HXa = sb_w.tile([128, F], f16, tag=f"hxa_{i % NB}",
                                        bufs=1, name="HXa")
                        HXb = sb_w.tile([128, F], f16, tag=f"hxb_{i % NB}",
                                        bufs=1, name="HXb")
                        HZ = sb_w.tile([96, F], f16, tag=f"hz_{i % NB}",
                                       bufs=1, name="HZ")
                        nc.sync.dma_start(HXa[:, 0:HALF], rep(16, 8, 16))
                        nc.sync.dma_start(HXa[:, HALF:F], rep(56, 8, 16))
                        nc.sync.dma_start(HXb[:, 0:HALF], rep(24, 8, 16))
                        nc.sync.dma_start(HXb[:, HALF:F], rep(64, 8, 16))
                        nc.sync.dma_start(HZ[:, 0:HALF], rep(32, 8, 12))
                        nc.sync.dma_start(HZ[:, HALF:F], rep(72, 8, 12))
                        hx[i] = (HXa, HXb, HZ)

                        HY = ps_hy.tile([128, F], f32, tag="hy", name="HY")
                        nc.tensor.matmul(HY[:, 0:HALF], st["stHYe"][:], nh[:],
                                         start=True, stop=True)
                        nc.tensor.matmul(HY[:, HALF:F], st["stHYo"][:], nh[:],
                                         start=True, stop=True)
                        HYS = sb_s.tile([128, F], f16, tag="hys", name="HYS")
                        nc.scalar.copy(HYS[:], HY[:])
                        Wa = sb_w.tile([128, F], f16, tag=f"wa_{i % NB}",
                                       bufs=1, name="Wa")
                        Wb = sb_w.tile([128, F], f16, tag=f"wb_{i % NB}",
                                       bufs=1, name="Wb")
                        nc.vector.tensor_tensor(out=Wa[:], in0=HYS[:], in1=HXa[:],
                                                op=mybir.AluOpType.mult)
                        nc.vector.tensor_tensor(out=Wb[:], in0=HYS[:], in1=HXb[:],
                                                op=mybir.AluOpType.mult)
                        wab[i] = (Wa, Wb)
                    for i in gtiles:      # phase 3: X96, CF, z-fold, out
                        IN6 = ins[i]
                        HXa, HXb, HZ = hx[i]
                        Wa, Wb = wab[i]
                        X96 = ps_x.tile([96, F], f32, tag="x96", name="X96")
                        nc.tensor.matmul(X96[:, 0:HALF], st["stXe"][:], IN6[:],
                                         start=True, stop=True)
                        nc.tensor.matmul(X96[:, HALF:F], st["stXo"][:], IN6[:],
                                         start=True, stop=True)
                        X96S = sb_s.tile([96, F], f16, tag="x96s", name="X96S")
                        nc.scalar.copy(X96S[:], X96[:])
                        HZX = sb_s.tile([96, F], f16, tag="hzx", name="HZX")
                        nc.vector.tensor_tensor(out=HZX[:], in0=X96S[:], in1=HZ[:],
                                                op=mybir.AluOpType.mult)
                        CF = ps_cf.tile([96, F], f32, tag="cf", name="CF")
                        nc.tensor.matmul(CF[:], st["Ga"][:], Wa[:], start=True, stop=False)
                        nc.tensor.matmul(CF[:], st["Gb"][:], Wb[:], start=False, stop=True)
                        M2 = sb_s.tile([96, F], f16, tag="m2", name="M2")
                        nc.vector.tensor_tensor(out=M2[:], in0=CF[:], in1=HZX[:],
                                                op=mybir.AluOpType.mult)
                        OUTP = ps_out.tile([6, HALF], f32, tag="outp", name="OUTP")
                        nc.tensor.matmul(OUTP[:], st["REDe"][:], M2[:, 0:HALF],
                                         start=True, stop=False)
                        nc.tensor.matmul(OUTP[:], st["REDo"][:], M2[:, HALF:F],
                                         start=False, stop=True)
                        OUTS = sb_s.tile([6, HALF], f16, tag="outs", name="OUTS")
                        nc.scalar.copy(OUTS[:], OUTP[:])
                        nc.sync.dma_start(out6.ap()[:, bass.ts(i, HALF)], OUTS[:])

            if hw_loop:
                with tc.For_i(0, reps) as _i:
                    body()
            else:
                for _ in range(reps):
                    body()

    nc.compile()
    return nc


def _pack_inputs(pixels, coords):
    """Full-size host packing -> per-core in6 arrays [12, T*HALF] f16."""
    p = np.asarray(pixels, np.float32).reshape(-1, 3)
    c = np.asarray(coords, np.float32).reshape(-1, 2)
    P6 = np.empty((6, NTOT), np.float16)
    P6[0] = p[:, 0]; P6[1] = p[:, 1]; P6[2] = p[:, 2]
    P6[3] = 1.0
    P6[4] = c[:, 1]          # cy
    P6[5] = c[:, 0]          # cx
    outs = []
    for cid in range(NCORES):
        s = P6[:, cid * NPC:(cid + 1) * NPC]
        in6 = np.ascontiguousarray(
            s.reshape(6, T, 2, HALF).transpose(2, 0, 1, 3).reshape(12, T * HALF))
        outs.append(in6)
    return outs


def kernel(pixels: np.ndarray, coords: np.ndarray, grid: np.ndarray) -> np.ndarray:
    assert pixels.shape == (B, H, W, 3) and coords.shape == (B, H, W, 2)
    stats = _make_stationaries(np.asarray(grid, np.float32))
    in6s = _pack_inputs(pixels, coords)
    in_maps = [{"in6": in6s[cid], **stats} for cid in range(NCORES)]

    if "nc" not in _CACHE:
        _CACHE["nc"] = build_kernel()
    nc = _CACHE["nc"]
    res = run_bass_kernel_spmd(nc, in_maps, list(range(NCORES)))
    parts = []
    for cid in range(NCORES):
        o6 = np.asarray(res.results[cid]["out6"])     # [6, T*HALF] f16
        o = o6.reshape(2, 3, T, HALF).transpose(2, 0, 3, 1).reshape(NPC, 3)
        parts.append(o)
    out = np.concatenate(parts, 0).astype(np.float32)
    return np.ascontiguousarray(out.reshape(B, H, W, 3))


# revision 52
# speedup vs baseline: 608.0423x; 5.8746x over previous
"""Bilateral grid slice+apply on 8 Trainium2 NeuronCores.

Gather-free hat-basis formulation, v4 (fp16, flat layout, minimal
broadcast descriptors):

    hat(a) = relu(1 - |a|);  nh = -hat  computed as  min(|a|-1, 0)
    W[cell] = hat_y * hat_x  (256 cells, x-major, two 128 halves)
    CF[z*12+k] = sum_cell W G[cell, z, k]         (PE, K=128 x2 accumulate)
    out_c = sum_p RED[p,c] * CF[p] * hz[p] * q[p]  (PE reduce K=96)

Inputs ride a flat [6, N] layout (r,g,b,1,cy,cx) so every per-tile
broadcast is ONE 3-dim DMA with 1 KiB per-partition descriptors:
  HXa/HXb [128,512] <- nh x-rows repeat-16, HZ [96,512] <- z-rows
  repeat-12 (352 descriptors/tile total, issued on the gpsimd SWDGE
  queue so their waits never block the IO queue).

Engine split per 512-px tile:
  PE   : args [6->40], HY one-hot [40->128], X96 one-hot [6->96],
         CFa+CFb (K=128 accum), OUT reduce [96->3]   (6 matmuls, fp16)
  ACT  : Abs(args+bias), HY->f16 copy, X96->f16 copy, OUT copy
  DVE  : nh chain (4x mode), Wab (one [128,1024] 2x mult), HZX, M2
  DMA  : 3 broadcasts/tile + paired IO
"""
import numpy as np
from contextlib import ExitStack

import concourse.bass as bass
import concourse.bacc as bacc
import concourse.mybir as mybir
from concourse import tile
from concourse.bass_utils import run_bass_kernel_spmd

F = 512             # pixels per tile
NCORES = 8
B, H, W = 4, 1080, 1920
NTOT = B * H * W                  # 8294400
NPC = NTOT // NCORES              # 1036800 per core
T = NPC // F                      # 2025 tiles per core
LUM = (0.2126, 0.7152, 0.0722)

_CACHE = {}


def _y_of_m(m):
    # cell slot -> y, from the 4x-duplicated x-arg broadcast pattern:
    # src row (4x + c) repeated 4 -> m = 4*(4x+c) + k, y = 4c + k
    return 4 * ((m % 16) // 4) + m % 4


def _make_stationaries(grid):
    """nh row layout: [0:16) y-args; [16:80) x-args (row 16+4x+c, c<4);
    [80:112) z-args (row 80+4z+c, c<4).  The x/z duplication spreads the
    broadcast DMA source reads over 4 SBUF AXI ports each."""
    g = grid.astype(np.float32)
    f16 = np.float16

    stARG = np.zeros((6, 112), np.float32)
    biasARG = np.zeros((112, 1), np.float32)
    for j in range(112):
        if j < 16:
            stARG[4, j] = 15.0      # gy = 15*cy
            biasARG[j, 0] = -j
        elif j < 80:
            stARG[5, j] = 15.0      # gx = 15*cx
            biasARG[j, 0] = -((j - 16) // 4)
        else:
            for c in range(3):
                stARG[c, j] = 7.0 * LUM[c]   # gz = 7*lum
            biasARG[j, 0] = -((j - 80) // 4)

    # HY one-hot: HY[m] = nh[y(m)]
    stHY = np.zeros((112, 128), np.float32)
    for m in range(128):
        stHY[_y_of_m(m), m] = 1.0

    # X96 one-hot: X96[p] = IN6[p%4]  (rows r,g,b,1)
    stX = np.zeros((6, 96), np.float32)
    for p in range(96):
        stX[p % 4, p] = 1.0

    # main grid stationaries: half a: x = m//16, half b: x = 8 + m//16
    Ga = np.zeros((128, 96), np.float32)
    Gb = np.zeros((128, 96), np.float32)
    for m in range(128):
        y, xp = _y_of_m(m), m // 16
        Ga[m, :] = g[y, xp].reshape(96)
        Gb[m, :] = g[y, 8 + xp].reshape(96)

    # final reduce (negated to absorb nh_z sign)
    RED = np.zeros((96, 3), np.float32)
    for p in range(96):
        RED[p, (p % 12) // 4] = -1.0

    return dict(
        stARG=stARG.astype(f16), biasARG=biasARG,
        stHY=stHY.astype(f16), stX=stX.astype(f16),
        Ga=Ga.astype(f16), Gb=Gb.astype(f16), RED=RED.astype(f16),
    )


STSHAPES = (("stARG", [6, 112], "f16"), ("biasARG", [112, 1], "f32"),
            ("stHY", [112, 128], "f16"), ("stX", [6, 96], "f16"),
            ("Ga", [128, 96], "f16"), ("Gb", [128, 96], "f16"),
            ("RED", [96, 3], "f16"))


def build_kernel(ntiles=T, num_cores=NCORES, reps=1, hw_loop=False,
                 grp=6, x1x=True, iopair=4, extra_dma=0, skeleton=False,
                 outs_dve=False, **_ignored):
    nc = bacc.Bacc("TRN2", target_bir_lowering=False, debug=False,
                   num_devices=num_cores)
    NP = ntiles * F
    f16, f32 = mybir.dt.float16, mybir.dt.float32
    DT = {"f16": f16, "f32": f32}

    in6 = nc.declare_dram_parameter("in6", [6, NP], f16, isOutput=False)
    decls = {nm: nc.declare_dram_parameter(nm, shp, DT[d], isOutput=False)
             for nm, shp, d in STSHAPES}
    out3 = nc.declare_dram_parameter("out3", [3, NP], f16, isOutput=True)

    with tile.TileContext(nc) as tc:
        with ExitStack() as ctx:
            stp = ctx.enter_context(tc.tile_pool(name="stats", bufs=1))
            st = {}
            for nm, shp, d in STSHAPES:
                t_ = stp.tile(shp, DT[d], tag=nm, name=nm)
                nc.sync.dma_start(t_[:], decls[nm].ap())
                st[nm] = t_

            sb_in = ctx.enter_context(tc.tile_pool(name="sb_in", bufs=1))
            sb_w = ctx.enter_context(tc.tile_pool(name="sb_w", bufs=1))
            sb_s = ctx.enter_context(tc.tile_pool(name="sb_s", bufs=3))
            ps_args = ctx.enter_context(tc.tile_pool(name="ps_args", bufs=2, space="PSUM"))
            ps_hy = ctx.enter_context(tc.tile_pool(name="ps_hy", bufs=2, space="PSUM"))
            ps_x = ctx.enter_context(tc.tile_pool(name="ps_x", bufs=2, space="PSUM"))
            ps_cf = ctx.enter_context(tc.tile_pool(name="ps_cf", bufs=2, space="PSUM"))

            G = grp
            NB = G + 1
            Abs = mybir.ActivationFunctionType.Abs
            mult = mybir.AluOpType.mult
            IOP = iopair            # tiles per IO DMA

            def load(i0, cnt):
                IN6 = sb_in.tile([6, cnt * F], f16, tag=f"in6_{(i0 // IOP) % NB}",
                                 bufs=1, name="IN6")
                nc.sync.dma_start(IN6[:], in6.ap()[:, i0 * F:(i0 + cnt) * F])
                return IN6

            def tile_body(i, IN6, c0, OUTS, oc0):
                """Full pipeline for tile i; IN6 cols c0:c0+F."""
                if skeleton:    # timing ablation: DMAs only, no compute
                    iap = IN6[:]
                    ip = iap.ap[0][0]
                    HXab = sb_w.tile([128, 2 * F], f16, tag=f"hxab_{i % NB}",
                                     bufs=1, name="HXab")
                    HZ = sb_w.tile([96, F], f16, tag=f"hz_{i % NB}",
                                   bufs=1, name="HZ")
                    src = bass.AP(iap.tensor, iap.offset + c0,
                                  [[ip, 2], [0, 64], [1, F]])
                    nc.gpsimd.dma_start(HXab[:, 0:F], src)
                    nc.gpsimd.dma_start(HXab[:, F:2 * F], src)
                    nc.gpsimd.dma_start(HZ[:], bass.AP(
                        iap.tensor, iap.offset + c0, [[ip, 2], [0, 48], [1, F]]))
                    nc.vector.tensor_scalar(out=OUTS[:, oc0:oc0 + F],
                                            in0=HZ[0:3, :], scalar1=1.0,
                                            scalar2=None,
                                            op0=mybir.AluOpType.mult)
                    return
                argsP = ps_args.tile([112, F], f32, tag="args", name="argsP")
                nc.tensor.matmul(argsP[:], st["stARG"][:], IN6[:, c0:c0 + F],
                                 start=True, stop=True)
                tabs = sb_s.tile([112, F], f16, tag="tabs", name="tabs")
                nc.scalar.activation(tabs[:], argsP[:], Abs,
                                     bias=st["biasARG"][:], scale=1.0)
                nh = sb_in.tile([112, F], f16, tag=f"nh_{i % NB}",
                                bufs=1, name="nh")
                nc.vector.tensor_scalar(out=nh[:], in0=tabs[:],
                                        scalar1=1.0, scalar2=0.0,
                                        op0=mybir.AluOpType.subtract,
                                        op1=mybir.AluOpType.min)
                nap = nh[:]
                ps = nap.ap[0][0]
                HXab = sb_w.tile([128, 2 * F], f16, tag=f"hxab_{i % NB}",
                                 bufs=1, name="HXab")
                HZ = sb_w.tile([96, F], f16, tag=f"hz_{i % NB}",
                               bufs=1, name="HZ")
                # one 3-dim broadcast DMA per tensor: 1KB descriptors,
                # sources spread over 4 AXI ports each (32 partitions)
                nc.gpsimd.dma_start(HXab[:, 0:F], bass.AP(
                    nap.tensor, nap.offset + 16 * ps, [[ps, 32], [0, 4], [1, F]]))
                nc.gpsimd.dma_start(HXab[:, F:2 * F], bass.AP(
                    nap.tensor, nap.offset + 48 * ps, [[ps, 32], [0, 4], [1, F]]))
                nc.gpsimd.dma_start(HZ[:], bass.AP(
                    nap.tensor, nap.offset + 80 * ps, [[ps, 32], [0, 3], [1, F]]))
                for d in range(extra_dma):
                    dum = sb_s.tile([8, 64], f16, tag=f"dum{d}", name="dum")
                    nc.gpsimd.dma_start(dum[:], bass.AP(
                        nap.tensor, nap.offset, [[ps, 8], [1, 64]]))

                HY = ps_hy.tile([128, F], f32, tag="hy", name="HY")
                nc.tensor.matmul(HY[:], st["stHY"][:], nh[:], start=True, stop=True)
                HYS = sb_s.tile([128, F], f16, tag="hys", name="HYS")
                nc.scalar.copy(HYS[:], HY[:])

                X96 = ps_x.tile([96, F], f32, tag="x96", name="X96")
                nc.tensor.matmul(X96[:], st["stX"][:], IN6[:, c0:c0 + F],
                                 start=True, stop=True)
                HZX = sb_s.tile([96, F], f16, tag="hzx", name="HZX")
                if x1x:
                    nc.vector.tensor_tensor(out=HZX[:], in0=X96[:], in1=HZ[:], op=mult)
                else:
                    X96S = sb_s.tile([96, F], f16, tag="x96s", name="X96S")
                    nc.scalar.copy(X96S[:], X96[:])
                    nc.vector.tensor_tensor(out=HZX[:], in0=X96S[:], in1=HZ[:], op=mult)

                Wab = sb_w.tile([128, 2 * F], f16, tag=f"wab_{i % NB}",
                                bufs=1, name="Wab")
                hap = HYS[:]
                hp = hap.ap[0][0]
                hrep = bass.AP(hap.tensor, hap.offset, [[hp, 128], [0, 2], [1, F]])
                nc.vector.tensor_tensor(out=Wab[:], in0=hrep, in1=HXab[:], op=mult)

                # OUT rides partitions 96-98 of the CF bank (M2 is both CF's
                # only reader and OUT's input, so no PSUM collision window)
                CFT = ps_cf.tile([128, F], f32, tag="cf", name="CFT")
                CF = CFT[0:96, :]
                nc.tensor.matmul(CF, st["Ga"][:], Wab[:, 0:F], start=True, stop=False)
                nc.tensor.matmul(CF, st["Gb"][:], Wab[:, F:2 * F], start=False, stop=True)
                M2 = sb_s.tile([96, F], f16, tag="m2", name="M2")
                nc.vector.tensor_tensor(out=M2[:], in0=CF, in1=HZX[:], op=mult)

                OUTP = CFT[96:99, :]
                nc.tensor.matmul(OUTP, st["RED"][:], M2[:], start=True, stop=True,
                                 tile_position=(0, 96), skip_group_check=True)
                if outs_dve:
                    nc.vector.tensor_scalar(out=OUTS[:, oc0:oc0 + F], in0=OUTP,
                                            scalar1=1.0, scalar2=None,
                                            op0=mybir.AluOpType.mult)
                else:
                    nc.scalar.copy(OUTS[:, oc0:oc0 + F], OUTP)

            def body():
                for g0 in range(0, ntiles, G * IOP):
                    blocks = []
                    for b0 in range(g0, min(g0 + G * IOP, ntiles), IOP):
                        cnt = min(IOP, ntiles - b0)
                        blocks.append((b0, cnt, load(b0, cnt)))
                    for b0, cnt, IN6 in blocks:
                        OUTS = sb_s.tile([3, cnt * F], f16, tag="outs", name="OUTS")
                        for k in range(cnt):
                            tile_body(b0 + k, IN6, k * F, OUTS, k * F)
                        nc.sync.dma_start(out3.ap()[:, b0 * F:(b0 + cnt) * F],
                                          OUTS[:])

            if hw_loop:
                with tc.For_i(0, reps) as _i:
                    body()
            else:
                for _ in range(reps):
                    body()

    nc.compile()
    return nc


def _pack_inputs(pixels, coords):
    """Full-size host packing -> per-core in6 arrays [6, NPC] f16."""
    p = np.asarray(pixels, np.float32).reshape(-1, 3)
    c = np.asarray(coords, np.float32).reshape(-1, 2)
    P6 = np.empty((6, NTOT), np.float16)
    P6[0] = p[:, 0]; P6[1] = p[:, 1]; P6[2] = p[:, 2]
    P6[3] = 1.0
    P6[4] = c[:, 1]          # cy
    P6[5] = c[:, 0]          # cx
    return [np.ascontiguousarray(P6[:, cid * NPC:(cid + 1) * NPC])
            for cid in range(NCORES)]


def kernel(pixels: np.ndarray, coords: np.ndarray, grid: np.ndarray) -> np.ndarray:
    assert pixels.shape == (B, H, W, 3) and coords.shape == (B, H, W, 2)
    stats = _make_stationaries(np.asarray(grid, np.float32))
    in6s = _pack_inputs(pixels, coords)
    in_maps = [{"in6": in6s[cid], **stats} for cid in range(NCORES)]

    if "nc" not in _CACHE:
        _CACHE["nc"] = build_kernel()
    nc = _CACHE["nc"]
    res = run_bass_kernel_spmd(nc, in_maps, list(range(NCORES)))
    parts = [np.asarray(res.results[cid]["out3"]).T for cid in range(NCORES)]
    out = np.concatenate(parts, 0).astype(np.float32)
    return np.ascontiguousarray(out.reshape(B, H, W, 3))
